# revision 29
# baseline (speedup 1.0000x reference)
"""Trainium2 Bass kernel for CausalAttention (sliding-window + scale-frame sparse attention).

Problem shape (hardcoded): B=1, N=4096, C=512, H=8, Dh=64, frame_seqlen=256,
sliding_window_size=2, num_frame_per_block=1, num_frame_for_scale=2.

Sharding: sequence-parallel over 8 NeuronCores. Core i owns queries
[512*i, 512*(i+1)) (= frames 2i, 2i+1) and returns that slice of the final
output. Keys needed per core: the 512 "scale" tokens (frames 0,1; attended by
every query unconditionally per the reference mask) plus a 3-frame window
{2i-1, 2i, 2i+1} (768 tokens). No collectives; host concatenates the slices.

Per-core device pipeline (all matmuls bf16 with fp32 PSUM accumulation):
  1. QKV projection: QT/KT = W @ x^T (channels on partitions, head pairs
     stacked 2x64 per 128-partition o-tile); V in natural [token, dh+1]
     layout with a per-head ones-column. Block-validity flags (scale overlap
     + window edge) are folded multiplicatively into the V evacuation (ACT
     copy with per-partition scale) and the ones-column (flag DMA), so they
     never touch the probabilities.
  2. Scores computed transposed, S^T[k, q] = K @ Q^T, per head into a single
     4-bank PSUM tile: first the 4 scale k-tiles (2048 cols), then the
     window k-tiles (1792 cols, kt7/kt9 trimmed to their causal q-range).
  3. One exp per pass on ScalarE straight out of PSUM (softmax scale folded
     into the activation scale; no max-subtraction: scores are O(10) so fp32
     exp cannot overflow).
  4. Pure-tril masks (identical on all cores) applied to the four diagonal
     sub-blocks of the window probabilities on DVE.
  5. O^T = V'^T @ P^T accumulated per head; row 64 holds softmax sums.
     Normalize via transpose-DMA + DVE reciprocal + GpSimd partition
     broadcast; odd heads' O^T is DMA-shifted to partitions 64:128 so head
     pairs stack to a 128-row tile.
  6. out^T = Wp_pair^T.T @ OT accumulated over the 4 pairs (contraction 128)
     at the tail, reusing the freed score banks; bias-add evac split across
     DVE+ACT, two output DMAs.
"""

from contextlib import ExitStack

import numpy as np
import ml_dtypes

N, C, H, DH = 4096, 512, 8, 64
F = 256                 # frame_seqlen
NCORES = 8
NQ = N // NCORES        # 512 queries per core (2 frames)
KS = 512                # scale tokens (frames 0,1)
KW = 3 * F              # window tokens per core
NK = KS + KW            # 1280 keys per core
BF16 = ml_dtypes.bfloat16
WARMUP = 24             # N=128 dummy matmuls to warm the HAM clock gate

# window-score chunk layouts (2-bank PSUM chunks; each matmul slice must
# stay inside one 512-col bank). Chunk WA = [kt6 q0:512 | kt7 q128:512],
# chunk WB = [kt4 q0:256 | kt5 q0:256 | kt8 q256:512 | kt9 q384:512].
WA_TILES = ((6, 0, 0, 512), (7, 512, 128, 512))
WB_TILES = ((4, 0, 0, 256), (5, 256, 0, 256), (8, 512, 256, 512),
            (9, 768, 384, 512))
WA_TOT, WB_TOT = 896, 896

_CACHE = {}


def _build(repeat=1):
    """Build + compile the (single, SPMD) Bass program. Returns nc."""
    import concourse.bass as bass  # noqa: F401
    import concourse.mybir as mybir
    import concourse.tile as tile
    from concourse import bacc

    f32 = mybir.dt.float32
    bf16 = mybir.dt.bfloat16
    EXP = mybir.ActivationFunctionType.Exp

    nc = bacc.Bacc("TRN2", target_bir_lowering=False, debug=False)

    xT = nc.dram_tensor("xT", [C, NK], bf16, kind="ExternalInput")
    wqT = nc.dram_tensor("wqT", [C, C], bf16, kind="ExternalInput")
    wkT = nc.dram_tensor("wkT", [C, C], bf16, kind="ExternalInput")
    wvT = nc.dram_tensor("wvT", [C, C], bf16, kind="ExternalInput")
    wp2 = nc.dram_tensor("wp2", [128, 4 * C], bf16, kind="ExternalInput")
    ident = nc.dram_tensor("ident", [128, 128], f32, kind="ExternalInput")
    ctab = nc.dram_tensor("ctab", [128, 24], f32, kind="ExternalInput")
    dmsk = nc.dram_tensor("dmsk", [128, 512], bf16, kind="ExternalInput")
    flcol = nc.dram_tensor("flcol", [128, 10 * H], bf16, kind="ExternalInput")
    outT = nc.dram_tensor("outT", [C, NQ], bf16, kind="ExternalOutput")

    with tile.TileContext(nc) as tc, ExitStack() as ctx:
        cp = ctx.enter_context(tc.tile_pool(name="const", bufs=1))
        dp = ctx.enter_context(tc.tile_pool(name="data", bufs=1))
        pp = ctx.enter_context(tc.tile_pool(name="pbuf", bufs=2))
        recp = ctx.enter_context(tc.tile_pool(name="rec", bufs=2))
        psp = ctx.enter_context(tc.tile_pool(name="ps", bufs=1, space="PSUM"))

        def body():
            # ---- inputs. xs arrives in token-column chunks so the V
            # projection can start on chunk 0 while the rest streams;
            # weights go on the scalar HWDGE queue in parallel. ----
            xs = cp.tile([128, 4, NK], bf16, tag="xs")
            xr = xT.ap().rearrange("(a p) t -> p a t", p=128)
            wv = cp.tile([128, 4, C], bf16, tag="wv")
            nc.scalar.dma_start(wv[:], wvT.ap().rearrange("(a p) o -> p a o", p=128))
            ct = cp.tile([128, 24], f32, tag="ct")
            nc.scalar.dma_start(ct[:], ctab.ap())
            fv = cp.tile([128, 10, H], bf16, tag="fv")
            nc.scalar.dma_start(fv[:], flcol.ap().rearrange("p (t h) -> p t h", h=H))
            for c0 in range(0, NK, 256):
                nc.sync.dma_start(xs[:, :, c0:c0 + 256], xr[:, :, c0:c0 + 256])
            wq = cp.tile([128, 4, C], bf16, tag="wq")
            nc.sync.dma_start(wq[:], wqT.ap().rearrange("(a p) o -> p a o", p=128))
            wk = cp.tile([128, 4, C], bf16, tag="wk")
            nc.sync.dma_start(wk[:], wkT.ap().rearrange("(a p) o -> p a o", p=128))
            wp = cp.tile([128, 4, C], bf16, tag="wp")
            nc.scalar.dma_start(wp[:], wp2.ap().rearrange("p (a o) -> p a o", a=4))
            dm = cp.tile([128, 512], bf16, tag="dm")
            nc.scalar.dma_start(dm[:], dmsk.ap())
            idn = cp.tile([128, 128], f32, tag="idn")
            nc.scalar.dma_start(idn[:], ident.ap())
            ones4 = cp.tile([1, 64], f32, tag="ones4")
            nc.vector.memset(ones4[:], 1.0)

            # ---- HAM warmup while inputs stream ----
            wmup = cp.tile([128, 128], bf16, tag="wmup")
            nc.vector.memset(wmup[:], 0.0)
            for i in range(WARMUP):
                ps = psp.tile([128, 512], f32, tag="qkv", bufs=2, name="wps")
                nc.tensor.matmul(ps[:, 0:128], lhsT=wmup[:], rhs=wmup[:],
                                 start=True, stop=True, skip_group_check=True)

            QT = dp.tile([128, 4, NQ], bf16, tag="QT")
            KT = dp.tile([128, 4, NK], bf16, tag="KT")
            V = dp.tile([128, 10, H, DH + 1], bf16, tag="V")
            OTs = dp.tile([128, 4, NQ], bf16, tag="OTs")
            oT = dp.tile([128, 4, NQ], bf16, tag="oT")

            # ---- V projection; flags folded in via the evac scale (split
            # across ACT+DVE, both idle here) + the ones-column flag write
            # (DVE: engine writes are element-exact; a DMA here would
            # read-modify-write neighboring V bytes) ----
            nc.vector.tensor_copy(V[:, :, :, DH:DH + 1], fv[:])
            for tt in range(10):
                ps = psp.tile([128, 512], f32, tag="qkv", bufs=2, name="vps")
                for ci in range(4):
                    nc.tensor.matmul(ps[:], lhsT=xs[:, ci, 128 * tt:128 * (tt + 1)],
                                     rhs=wv[:, ci, :], start=(ci == 0), stop=(ci == 3))
                if tt % 2 == 0:
                    nc.scalar.mul(V[:, tt, :, 0:DH],
                                  ps[:].rearrange("p (h d) -> p h d", h=H),
                                  ct[:, 12 + tt:13 + tt])
                else:
                    nc.vector.tensor_scalar_mul(
                        V[:, tt, :, 0:DH],
                        ps[:].rearrange("p (h d) -> p h d", h=H),
                        ct[:, 12 + tt:13 + tt])

            def qk_proj(p):
                ps = psp.tile([128, 512], f32, tag="qkv", bufs=2, name="qps")
                for ci in range(4):
                    nc.tensor.matmul(ps[:], lhsT=wq[:, ci, 128 * p:128 * (p + 1)],
                                     rhs=xs[:, ci, KS + F:KS + F + NQ],
                                     start=(ci == 0), stop=(ci == 3))
                nc.vector.tensor_scalar_add(QT[:, p, :], ps[:], ct[:, p:p + 1])
                for t0, t1 in ((0, 512), (512, 1024), (1024, 1280)):
                    ps = psp.tile([128, 512], f32, tag="qkv", bufs=2, name="kps")
                    for ci in range(4):
                        nc.tensor.matmul(ps[:, 0:t1 - t0],
                                         lhsT=wk[:, ci, 128 * p:128 * (p + 1)],
                                         rhs=xs[:, ci, t0:t1],
                                         start=(ci == 0), stop=(ci == 3))
                    nc.vector.tensor_scalar_add(KT[:, p, t0:t1], ps[:, 0:t1 - t0],
                                                ct[:, 4 + p:5 + p])

            def norm_chain(h, av, fast=False):
                """softmax-normalize av[0:64] -> OTs (odd heads shifted).

                fast=True (tail heads) replaces the two transpose-DMAs +
                GpSimd broadcast (~2us completion receipt each) with PE
                transposes and a ones-broadcast matmul: all engine-local,
                ~3us lower latency."""
                po, odd = h // 2, h % 2
                sm = recp.tile([65, NQ], f32, tag="sm", name="sm")
                nc.vector.tensor_copy(sm[:], av[0:65, :])
                if fast:
                    rsP = psp.tile([128, 4], f32, tag="av", bufs=2,
                                   name="rsP")
                    for t in range(4):
                        nc.tensor.transpose(rsP[:, t:t + 1],
                                            sm[64:65, 128 * t:128 * (t + 1)],
                                            idn[64:65, 64:65])
                    rsr = recp.tile([128, 4], f32, tag="rs", name="rsr")
                    nc.vector.reciprocal(rsr[:], rsP[:])
                    rrowP = psp.tile([1, NQ], f32, tag="qkv", bufs=2,
                                     name="rrowP")
                    for t in range(4):
                        nc.tensor.transpose(rrowP[0:1, 128 * t:128 * (t + 1)],
                                            rsr[:, t:t + 1], idn[:])
                    rrow = recp.tile([1, NQ], f32, tag="rrowf", name="rrowf")
                    nc.scalar.copy(rrow[:], rrowP[:])
                    rcb = psp.tile([64, NQ], f32, tag="qkv", bufs=2,
                                   name="rcb")
                    nc.tensor.matmul(rcb[:], lhsT=ones4[0:1, :],
                                     rhs=rrow[:], start=True, stop=True,
                                     skip_group_check=True)
                else:
                    rs = recp.tile([128, 4], f32, tag="rs", name="rs")
                    nc.sync.dma_start(rs[:], sm[64:65, :])
                    nc.vector.reciprocal(rs[:], rs[:])
                    rrow = recp.tile([1, NQ], f32, tag="rrow", name="rrow")
                    nc.sync.dma_start(rrow[:], rs[:])
                    rcb = recp.tile([64, NQ], f32, tag="rcb", name="rcb")
                    nc.gpsimd.partition_broadcast(rcb[:], rrow[:])
                if not odd:
                    nc.vector.tensor_mul(OTs[0:64, po, :], sm[0:64, :],
                                         rcb[0:64, :])
                else:
                    ot_t = recp.tile([64, NQ], bf16, tag="ot_t", name="ot_t")
                    nc.vector.tensor_mul(ot_t[:], sm[0:64, :], rcb[0:64, :])
                    nc.sync.dma_start(OTs[64:128, po, :], ot_t[:])

            # ---- attention: per head, four double-buffered 2-bank score
            # chunks (scale A/B, window A/B) so PE matmuls and ACT exps
            # pipeline; the previous head's window-AV + normalize and the
            # next pair's QK projection fill the exp latencies. Odd heads
            # run first within each pair so the final head (h6) needs no
            # OT partition-shift DMA on the tail.
            scal = float(DH) ** -0.5
            prev = None     # (h, av, PWA, PWB) awaiting window-AV + norm

            def av_win_mms(h, av, PWA, PWB):
                for kt, w0, q0, q1 in WA_TILES:
                    nc.tensor.matmul(av[0:65, q0:q1], lhsT=V[:, kt, h, :],
                                     rhs=PWA[:, w0:w0 + q1 - q0],
                                     start=False, stop=False,
                                     skip_group_check=True)
                for j, (kt, w0, q0, q1) in enumerate(WB_TILES):
                    nc.tensor.matmul(av[0:65, q0:q1], lhsT=V[:, kt, h, :],
                                     rhs=PWB[:, w0:w0 + q1 - q0],
                                     start=False, stop=(j == 3),
                                     skip_group_check=True)

            def av_win(h, av, PWA, PWB):
                av_win_mms(h, av, PWA, PWB)
                norm_chain(h, av)

            qk_proj(0)
            for po in range(4):
                heads = (2 * po + 1, 2 * po)    # odd first: no tail OT shift
                kq = [(KT[64:128, po, :], QT[64:128, po, :]),
                      (KT[0:64, po, :], QT[0:64, po, :])]

                def sc_chunk(tiles, name):
                    """Emit one score chunk for both heads, matmul-paired so
                    the (64,0)/(0,0) row groups run concurrently."""
                    s1 = psp.tile([128, 1024], f32, tag="sc", bufs=2,
                                  name=name, uniquify=True)
                    s0 = psp.tile([128, 1024], f32, tag="sc", bufs=2,
                                  name=name, uniquify=True)
                    for kt, w0, q0, q1 in tiles:
                        for s, (kh, qh) in zip((s1, s0), kq):
                            nc.tensor.matmul(s[:, w0:w0 + q1 - q0],
                                             lhsT=kh[:, 128 * kt:128 * (kt + 1)],
                                             rhs=qh[:, q0:q1],
                                             start=True, stop=True)
                    tot = tiles[-1][1] + tiles[-1][3] - tiles[-1][2]
                    out = []
                    for s in (s1, s0):
                        P = pp.tile([128, 1024], bf16, tag="P", bufs=8,
                                    name="P", uniquify=True)
                        nc.scalar.activation(P[:, 0:tot], s[:, 0:tot],
                                             EXP, scale=scal)
                        out.append(P)
                    return out

                PA1, PA0 = sc_chunk(((0, 0, 0, 512), (1, 512, 0, 512)), "scA")
                PB1, PB0 = sc_chunk(((2, 0, 0, 512), (3, 512, 0, 512)), "scB")

                # fill the exp latency with the previous pair's window-AV,
                # normalize chains
                if prev is not None:
                    av_win(*prev[0])
                    av_win(*prev[1])

                PWA1, PWA0 = sc_chunk(WA_TILES, "swA")
                PWB1, PWB0 = sc_chunk(WB_TILES, "swB")

                # pure-tril masks on the diagonal sub-blocks
                for PWA, PWB in ((PWA1, PWB1), (PWA0, PWB0)):
                    nc.vector.tensor_mul(PWA[:, 0:256], PWA[:, 0:256],
                                         dm[:, 0:256])
                    nc.vector.tensor_mul(PWA[:, 512:640], PWA[:, 512:640],
                                         dm[:, 384:512])
                    nc.vector.tensor_mul(PWB[:, 512:768], PWB[:, 512:768],
                                         dm[:, 0:256])
                    nc.vector.tensor_mul(PWB[:, 768:896], PWB[:, 768:896],
                                         dm[:, 384:512])

                # O^T scale-part accumulation (sums in row 64 via ones col)
                avs = []
                for hi, h in enumerate(heads):
                    av = psp.tile([128, 512], f32, tag="av", bufs=2, name="av")
                    PA, PB = (PA1, PB1) if hi == 0 else (PA0, PB0)
                    for kt in range(4):
                        P = PA if kt < 2 else PB
                        nc.tensor.matmul(
                            av[0:65, :], lhsT=V[:, kt, h, :],
                            rhs=P[:, 512 * (kt % 2):512 * (kt % 2 + 1)],
                            start=(kt == 0), stop=False, skip_group_check=True)
                    avs.append(av)
                # fill the window-exp latency with the next pair's projection
                if po < 3:
                    qk_proj(po + 1)
                prev = ((heads[0], avs[0], PWA1, PWB1),
                        (heads[1], avs[1], PWA0, PWB0))
            av_win_mms(*prev[0])
            av_win_mms(*prev[1])
            norm_chain(prev[0][0], prev[0][1], fast=True)
            norm_chain(prev[1][0], prev[1][1], fast=True)

            # ---- output projection, reusing the freed score banks.
            # Pairs 0-2 contract 128 rows at once; pair 3 is split per head
            # so h7's contribution (ready early) doesn't wait for h6's
            # normalize chain, which is the tail.
            pjA = psp.tile([128, 2, 512], f32, tag="sc", bufs=2, name="pjA")
            pjB = psp.tile([128, 2, 512], f32, tag="sc", bufs=2, name="pjB")

            def pj_mm(ot, lhsT, rhs, start, stop):
                pj = pjA if ot < 2 else pjB
                nc.tensor.matmul(pj[:, ot % 2, :], lhsT=lhsT, rhs=rhs,
                                 start=start, stop=stop,
                                 skip_group_check=True)

            for hp in range(4):
                for ot in range(4):
                    pj_mm(ot, wp[:, hp, 128 * ot:128 * (ot + 1)],
                          OTs[:, hp, :], hp == 0, hp == 3)
            od = outT.ap().rearrange("(a p) q -> p a q", p=128)
            IDENT = mybir.ActivationFunctionType.Identity
            for ot in (0, 2, 1, 3):
                pj = (pjA if ot < 2 else pjB)[:, ot % 2, :]
                if ot % 2 == 0:
                    nc.vector.tensor_scalar_add(oT[:, ot, :], pj,
                                                ct[:, 8 + ot:9 + ot])
                    nc.sync.dma_start(od[:, ot, :], oT[:, ot, :])
                else:
                    nc.scalar.activation(oT[:, ot, :], pj, IDENT,
                                         bias=ct[:, 8 + ot:9 + ot])
                    nc.scalar.dma_start(od[:, ot, :], oT[:, ot, :])

        if repeat == 1:
            body()
        else:
            with tc.For_i(0, repeat, 1):
                body()

    nc.compile()
    return nc


def _get_nc(repeat=1):
    key = ("nc", repeat)
    if key not in _CACHE:
        _CACHE[key] = _build(repeat)
    return _CACHE[key]


def _host_prep(x, qkv_w, qkv_b, proj_w, proj_b):
    """Build the 8 per-core input maps."""
    x = np.asarray(x, np.float32).reshape(N, C)
    qkv_w = np.asarray(qkv_w, np.float32)
    qkv_b = np.asarray(qkv_b, np.float32)
    proj_w = np.asarray(proj_w, np.float32)
    proj_b = np.asarray(proj_b, np.float32)

    xs_bf = x.astype(BF16)
    xT_scale = np.ascontiguousarray(xs_bf[0:KS].T)            # [C, 512]
    wqT = np.ascontiguousarray(qkv_w[0:C].T.astype(BF16))
    wkT = np.ascontiguousarray(qkv_w[C:2 * C].T.astype(BF16))
    wvT = np.ascontiguousarray(qkv_w[2 * C:3 * C].T.astype(BF16))
    # pair-stacked proj weight: wp2[d, p*512+o] = proj_w[o, 128p + d]
    wp2 = np.ascontiguousarray(
        proj_w.T.reshape(4, 128, C).transpose(1, 0, 2).reshape(128, 4 * C)
        .astype(BF16))

    # value-bias folds through normalized attention into the proj bias:
    # O = sum_k phat_k (V_k + vb) = O_hat + vb, so out += vb @ proj_w.T
    pb_eff = proj_b + qkv_b[2 * C:3 * C] @ proj_w.T
    # tril01[j, q] = 1 if key j <= query q (within the same frame)
    tril01 = (np.arange(F)[:, None] <= np.arange(F)[None, :]).astype(np.float32)
    dmsk = np.empty((128, 512), np.float32)
    dmsk[:, 0:256] = tril01[0:128, :]          # tril_a (kt6/kt8 patterns)
    dmsk[:, 256:512] = tril01[128:256, :]      # tril_b; [:,384:512] = kt7/kt9

    in_maps = []
    for i in range(NCORES):
        win = np.zeros((KW, C), BF16)
        lo = F * (2 * i - 1)
        src = xs_bf[max(0, lo):F * (2 * i + 2)]
        win[KW - len(src):] = src
        xTi = np.empty((C, NK), BF16)
        xTi[:, 0:KS] = xT_scale
        xTi[:, KS:] = win.T

        vf0 = 1.0 if (2 * i - 1) >= 2 else 0.0
        vd0 = 1.0 if (2 * i) >= 2 else 0.0
        vd1 = 1.0 if (2 * i + 1) >= 2 else 0.0
        fl = np.array([1, 1, 1, 1, vf0, vf0, vd0, vd0, vd1, vd1], np.float32)

        ctab = np.zeros((128, 24), np.float32)
        for ot in range(4):
            ctab[:, ot] = qkv_b[0:C][128 * ot:128 * (ot + 1)]
            ctab[:, 4 + ot] = qkv_b[C:2 * C][128 * ot:128 * (ot + 1)]
            ctab[:, 8 + ot] = pb_eff[128 * ot:128 * (ot + 1)]
        ctab[:, 12:22] = fl[None, :]

        flcol = np.broadcast_to(fl[None, :, None], (128, 10, H))
        in_maps.append({
            "xT": xTi, "wqT": wqT, "wkT": wkT, "wvT": wvT, "wp2": wp2,
            "ident": np.eye(128, dtype=np.float32),
            "ctab": ctab, "dmsk": dmsk.astype(BF16),
            "flcol": np.ascontiguousarray(flcol.reshape(128, 10 * H)
                                          .astype(BF16)),
        })
    return in_maps


def _check_fixed_params(block_mask, video_mask, frame_seqlen,
                        sliding_window_size, num_frame_per_block,
                        num_frame_for_scale):
    if int(frame_seqlen) != F or int(sliding_window_size) != 2 \
            or int(num_frame_per_block) != 1 or int(num_frame_for_scale) != 2:
        return False
    vm = np.asarray(video_mask)
    if not bool(vm.all()):
        return False
    bm = np.asarray(block_mask)
    if bm.shape != (N, N):
        return False
    # spot-check causality structure of block_mask (full check is 16M bools)
    idx = np.linspace(0, N - 1, 64).astype(int)
    sub = bm[np.ix_(idx, idx)]
    if not np.array_equal(sub, np.tril(np.ones_like(sub))):
        return False
    return True


def _numpy_reference(x, block_mask, video_mask, qkv_w, qkv_b, proj_w, proj_b,
                     frame_seqlen, sliding_window_size, num_frame_per_block,
                     num_frame_for_scale):
    """Fallback: direct numpy evaluation of the reference semantics."""
    x = np.asarray(x, np.float32)
    b, n, c = x.shape
    dh = c // H
    qkv = (x @ np.asarray(qkv_w).T + np.asarray(qkv_b)).reshape(b, n, 3, H, dh)
    qkv = qkv.transpose(2, 0, 3, 1, 4)
    q, k, v = qkv[0], qkv[1], qkv[2]
    mask = np.asarray(block_mask)[:n, :n][None, None]
    vm = np.asarray(video_mask)[:, None, None, None]
    mask = mask | ~vm
    fs = int(frame_seqlen)
    if int(sliding_window_size) > 0 and fs is not None:
        f = np.arange(n) // fs
        w = int(sliding_window_size) * int(num_frame_per_block)
        sliding = (f[None, :] <= f[:, None]) & (f[None, :] >= f[:, None] - w + 1)
        mask = mask & sliding[None, None]
        if int(num_frame_for_scale) > 0:
            s = int(num_frame_for_scale) * fs
            mask = mask.copy()
            mask[:, :, :, :s] = True
    scores = np.einsum('bhqd,bhkd->bhqk', q, k) * (dh ** -0.5)
    scores = np.where(mask, scores, np.float32(-1e30))
    scores -= scores.max(axis=-1, keepdims=True)
    e = np.exp(scores)
    attn = e / e.sum(axis=-1, keepdims=True)
    o = np.einsum('bhqk,bhkd->bhqd', attn, v)
    o = o.transpose(0, 2, 1, 3).reshape(b, n, c)
    return (o @ np.asarray(proj_w).T + np.asarray(proj_b)).astype(np.float32)


def kernel(x, block_mask, video_mask, qkv_w, qkv_b, proj_w, proj_b,
           frame_seqlen, sliding_window_size, num_frame_per_block,
           num_frame_for_scale):
    if not _check_fixed_params(block_mask, video_mask, frame_seqlen,
                               sliding_window_size, num_frame_per_block,
                               num_frame_for_scale):
        return _numpy_reference(x, block_mask, video_mask, qkv_w, qkv_b,
                                proj_w, proj_b, frame_seqlen,
                                sliding_window_size, num_frame_per_block,
                                num_frame_for_scale)

    from concourse.bass_utils import run_bass_kernel_spmd

    nc = _get_nc()
    in_maps = _host_prep(x, qkv_w, qkv_b, proj_w, proj_b)
    res = run_bass_kernel_spmd(nc, in_maps, core_ids=list(range(NCORES)))
    out = np.empty((N, C), np.float32)
    for i in range(NCORES):
        out[NQ * i:NQ * (i + 1)] = np.asarray(res.results[i]["outT"], np.float32).T
    return out.reshape(1, N, C)
